# revision 11
# baseline (speedup 1.0000x reference)
"""Trainium2 Bass kernel for nn_AttrAttentionLayer (GAT-style attention layer).

Reference computation per batch element b (N=2048 nodes, F_in=256, F_out=64):
    Wh = h @ W                                  [N, F_out]
    f1 = Wh @ a1 ; f2 = Wh @ a2                 [N]
    e  = leaky_relu(f1[:,None] + f2[None,:], 0.2) * node_type
    att= softmax(where(adj>0, e, -9e15), axis=0)   (softmax over i, per column j)
    out= relu(att @ (Wh * level[:,None]))       [N, F_out]

Sharding: batch dim B=8 -> one batch element per NeuronCore (pure data
parallel, no collectives).

Per-core algorithm (all scores kept on-chip, adj/node_type streamed once):
  - Wh/f1/f2 via PE matmuls from on-chip-transposed h tiles.
  - Scores computed in natural [i_part, j_free] layout:
        s = Lrelu(F2bc + f1_i)      (ScalarE, per-partition bias)
        s = s * node_type           (VectorE)
        s = (s + 60) * float(adj)   (VectorE scalar_tensor_tensor; adj
                                     converted int32->f32 on GpSimd)
    so masked entries become exactly 0 and unmasked (s+60).
  - PE transposes 128x128 score blocks into PSUM; ScalarE evicts with
    Exp(x - 60) into a resident bf16 P^T [j_part, i_free] (8MB), also
    emitting per-column sums via accum_out.  exp(0-60)~8.8e-27 reproduces
    the reference's exp(-9e15 - max) = 0 for masked entries.
  - whl[j,o] = Wh[j,o] * level[j] / colsum[j]  (bf16)
  - h'^T[o,i] = sum_j whl[j,o] * P^T[j,i]  (PSUM-accumulated matmuls),
    Relu on evict; final PE transpose back to [i, o].
"""

import os
import sys

import numpy as np

_REPO = "/opt/trn_rl_repo"
if _REPO not in sys.path:
    sys.path.insert(0, _REPO)

import concourse.bass as bass  # noqa: E402
import concourse.tile as tile  # noqa: E402
from concourse import bacc, masks, mybir  # noqa: E402

FP32 = mybir.dt.float32
BF16 = mybir.dt.bfloat16
I32 = mybir.dt.int32

ALPHA = 0.2
MASK_SHIFT = 60.0


class Cfg:
    def __init__(self, N=2048, F_in=256, F_out=64):
        assert N % 128 == 0 and F_in % 128 == 0
        self.N, self.F_in, self.F_out = N, F_in, F_out
        self.NTI = N // 128            # i/j tiles of 128 rows
        self.GRP = min(4, self.NTI)    # i-tiles per transpose group
        assert self.NTI % self.GRP == 0
        self.NIG = self.NTI // self.GRP
        self.IC = 128 * self.GRP       # i-chunk (psum free dim), <= 512
        self.NFC = F_in // 128         # f-blocks of contraction dim


def attn_kernel(tc: tile.TileContext, out_ap, in_aps, cfg: Cfg):
    """Emit the per-core kernel. in_aps: dict name -> bass.AP."""
    from contextlib import ExitStack

    nc = tc.nc
    N, F_in, F_out = cfg.N, cfg.F_in, cfg.F_out
    NTI, GRP, NIG, IC, NFC = cfg.NTI, cfg.GRP, cfg.NIG, cfg.IC, cfg.NFC

    h_d = in_aps["h"]
    adj_d = in_aps["adj"]
    nt_d = in_aps["node_type"]
    level_d = in_aps["level"]
    W_d = in_aps["W"]
    a_d = in_aps["a"]

    with ExitStack() as ctx:
        # ---------- persistent SBUF ----------
        persist = ctx.enter_context(tc.tile_pool(name="persist", bufs=1))
        id128 = persist.tile([128, 128], FP32, tag="id128")
        masks.make_identity(nc, id128[:])

        pt_all = persist.tile([128, NTI, N], BF16, tag="pt")      # P^T tiles
        f2bc = persist.tile([128, N], FP32, tag="f2bc")           # f2 bcast
        f1_all = persist.tile([128, NTI], FP32, tag="f1")         # f1 cols
        f2row = persist.tile([1, N], FP32, tag="f2row")
        wh_all = persist.tile([128, NTI * F_out], FP32, tag="wh")
        whl2 = persist.tile([128, NTI * F_out], BF16, tag="whl2")
        cs_parts = persist.tile([128, NTI * NIG], FP32, tag="csp")
        cs = persist.tile([128, NTI], FP32, tag="cs")
        inv_cs = persist.tile([128, NTI], FP32, tag="invcs")
        level_sb = persist.tile([128, NTI], FP32, tag="level")
        hpT = persist.tile([F_out, N], FP32, tag="hpT")           # h'^T
        W_sb = persist.tile([128, NFC, F_out], FP32, tag="W")
        a1_sb = persist.tile([F_out, 1], FP32, tag="a1")
        a2_sb = persist.tile([F_out, 1], FP32, tag="a2")
        ones_sb = persist.tile([1, 128], FP32, tag="ones")
        nc.vector.memset(ones_sb[:], 1.0)
        neg_shift = persist.tile([128, 1], FP32, tag="negshift")
        nc.vector.memset(neg_shift[:], -MASK_SHIFT)

        # level[j]: tile tj's rows as column tj -> rearrange "(t p) -> p t"
        nc.sync.dma_start(out=level_sb[:, :],
                          in_=level_d.rearrange("(t p) -> p t", p=128))
        for c in range(NFC):
            nc.sync.dma_start(out=W_sb[:, c, :],
                              in_=W_d[c * 128:(c + 1) * 128, :])
        nc.sync.dma_start(out=a1_sb[:, :], in_=a_d[:F_out, :])
        nc.sync.dma_start(out=a2_sb[:, :], in_=a_d[F_out:, :])

        # ---------- phase 1: Wh, f1, f2 ----------
        with ExitStack() as p1:
            sb1 = p1.enter_context(tc.tile_pool(name="sb1", bufs=2))
            psA = p1.enter_context(tc.tile_pool(name="psA", bufs=2, space="PSUM"))
            psB = p1.enter_context(tc.tile_pool(name="psB", bufs=1, space="PSUM"))
            for ti in range(NTI):
                h_t = sb1.tile([128, F_in], FP32, tag="h")
                nc.sync.dma_start(out=h_t[:], in_=h_d[ti * 128:(ti + 1) * 128, :])
                hT_ps = psA.tile([128, F_in], FP32, tag="hT")
                for c in range(NFC):
                    nc.tensor.transpose(hT_ps[:, c * 128:(c + 1) * 128],
                                        h_t[:, c * 128:(c + 1) * 128], id128[:])
                hT_sb = sb1.tile([128, F_in], FP32, tag="hTsb")
                nc.vector.tensor_copy(hT_sb[:], hT_ps[:])

                wh_ps = psA.tile([128, F_out], FP32, tag="whps")
                whT_ps = psB.tile([F_out, 128], FP32, tag="whTps")
                for c in range(NFC):
                    blk = hT_sb[:, c * 128:(c + 1) * 128]
                    nc.tensor.matmul(wh_ps[:], blk, W_sb[:, c, :],
                                     start=(c == 0), stop=(c == NFC - 1))
                    nc.tensor.matmul(whT_ps[:], W_sb[:, c, :], blk,
                                     start=(c == 0), stop=(c == NFC - 1))
                nc.vector.tensor_copy(wh_all[:, ti * F_out:(ti + 1) * F_out],
                                      wh_ps[:])
                whT_sb = sb1.tile([F_out, 128], FP32, tag="whTsb")
                nc.vector.tensor_copy(whT_sb[:], whT_ps[:])

                f1_ps = psB.tile([128, 1], FP32, tag="f1ps")
                nc.tensor.matmul(f1_ps[:], whT_sb[:], a1_sb[:],
                                 start=True, stop=True)
                nc.vector.tensor_copy(f1_all[:, ti:ti + 1], f1_ps[:])
                f2_ps = psB.tile([1, 128], FP32, tag="f2ps")
                nc.tensor.matmul(f2_ps[:], a2_sb[:], whT_sb[:],
                                 start=True, stop=True)
                nc.vector.tensor_copy(f2row[:, ti * 128:(ti + 1) * 128], f2_ps[:])

        # ---------- phase 2: broadcast f2 row across partitions ----------
        with ExitStack() as p2:
            psBC = p2.enter_context(tc.tile_pool(name="psBC", bufs=2, space="PSUM"))
            for c0 in range(0, N, 512):
                w = min(512, N - c0)
                bc_ps = psBC.tile([128, 512], FP32, tag="bc")
                nc.tensor.matmul(bc_ps[:, :w], ones_sb[:],
                                 f2row[:, c0:c0 + w], start=True, stop=True)
                nc.vector.tensor_copy(f2bc[:, c0:c0 + w], bc_ps[:, :w])

        # ---------- phases 3-5 ----------
        with ExitStack() as p3:
            io = p3.enter_context(tc.tile_pool(name="io", bufs=2))
            cvt = p3.enter_context(tc.tile_pool(name="cvt", bufs=2))
            sc = p3.enter_context(tc.tile_pool(name="sc", bufs=GRP + 1))
            ps_tp = p3.enter_context(tc.tile_pool(name="pstp", bufs=3, space="PSUM"))
            ps_mm = p3.enter_context(tc.tile_pool(name="psmm", bufs=2, space="PSUM"))
            out_pool = p3.enter_context(tc.tile_pool(name="outp", bufs=2))

            # phase 3: scores -> P^T (bf16) + column-sum partials
            for ig in range(NIG):
                s_grp = []
                for q in range(GRP):
                    ti = ig * GRP + q
                    adj_t = io.tile([128, N], I32, tag="adj")
                    nc.sync.dma_start(out=adj_t[:],
                                      in_=adj_d[ti * 128:(ti + 1) * 128, :])
                    nt_t = io.tile([128, N], FP32, tag="nt")
                    nc.sync.dma_start(out=nt_t[:],
                                      in_=nt_d[ti * 128:(ti + 1) * 128, :])
                    adjf_t = cvt.tile([128, N], FP32, tag="adjf")
                    nc.gpsimd.tensor_copy(adjf_t[:], adj_t[:])

                    s_t = sc.tile([128, N], FP32, tag="score")
                    # u = f2bc + f1  (ScalarE), then leaky-relu as
                    # max(0.2*u, u) in one fused VectorE op.
                    nc.scalar.activation(s_t[:], f2bc[:],
                                         mybir.ActivationFunctionType.Identity,
                                         bias=f1_all[:, ti:ti + 1], scale=1.0)
                    nc.vector.scalar_tensor_tensor(
                        out=s_t[:], in0=s_t[:], scalar=ALPHA, in1=s_t[:],
                        op0=mybir.AluOpType.mult, op1=mybir.AluOpType.max)
                    nc.vector.tensor_mul(s_t[:], s_t[:], nt_t[:])
                    nc.vector.scalar_tensor_tensor(
                        out=s_t[:], in0=s_t[:], scalar=MASK_SHIFT, in1=adjf_t[:],
                        op0=mybir.AluOpType.add, op1=mybir.AluOpType.mult)
                    s_grp.append(s_t)

                for tj in range(NTI):
                    tp_ps = ps_tp.tile([128, IC], FP32, tag="tp")
                    for q in range(GRP):
                        nc.tensor.transpose(tp_ps[:, q * 128:(q + 1) * 128],
                                            s_grp[q][:, tj * 128:(tj + 1) * 128],
                                            id128[:])
                    nc.scalar.activation(pt_all[:, tj, ig * IC:(ig + 1) * IC],
                                         tp_ps[:],
                                         mybir.ActivationFunctionType.Exp,
                                         bias=neg_shift[:], scale=1.0,
                                         accum_out=cs_parts[:, tj * NIG + ig:
                                                            tj * NIG + ig + 1])

            # colsums -> whl2
            nc.vector.tensor_reduce(
                cs[:], cs_parts[:].rearrange("p (t g) -> p t g", g=NIG),
                axis=mybir.AxisListType.X, op=mybir.AluOpType.add)
            nc.vector.reciprocal(inv_cs[:], cs[:])
            for tj in range(NTI):
                nc.vector.tensor_scalar(
                    out=whl2[:, tj * F_out:(tj + 1) * F_out],
                    in0=wh_all[:, tj * F_out:(tj + 1) * F_out],
                    scalar1=level_sb[:, tj:tj + 1],
                    scalar2=inv_cs[:, tj:tj + 1],
                    op0=mybir.AluOpType.mult, op1=mybir.AluOpType.mult)

            # phase 4: h'^T[o, i] = sum_j whl2[j, o] * P^T[j, i], relu on evict
            for ic in range(N // IC):
                mm_ps = ps_mm.tile([F_out, IC], FP32, tag="mm")
                for tj in range(NTI):
                    nc.tensor.matmul(mm_ps[:],
                                     whl2[:, tj * F_out:(tj + 1) * F_out],
                                     pt_all[:, tj, ic * IC:(ic + 1) * IC],
                                     start=(tj == 0), stop=(tj == NTI - 1))
                nc.scalar.activation(hpT[:, ic * IC:(ic + 1) * IC], mm_ps[:],
                                     mybir.ActivationFunctionType.Relu)

            # phase 5: transpose h'^T -> [i, o], DMA out
            for ti in range(NTI):
                ot_ps = ps_tp.tile([128, F_out], FP32, tag="ot")
                nc.tensor.transpose(ot_ps[:], hpT[:, ti * 128:(ti + 1) * 128],
                                    id128[:F_out, :F_out])
                o_sb = out_pool.tile([128, F_out], FP32, tag="osb")
                nc.vector.tensor_copy(o_sb[:], ot_ps[:])
                nc.sync.dma_start(out=out_ap[ti * 128:(ti + 1) * 128, :],
                                  in_=o_sb[:])


def build(cfg: Cfg):
    """Build the single-core Bass program (same program for all cores)."""
    nc = bacc.Bacc("TRN2", target_bir_lowering=False, debug=False)
    N, F_in, F_out = cfg.N, cfg.F_in, cfg.F_out
    in_aps = {
        "h": nc.dram_tensor("h", [N, F_in], FP32, kind="ExternalInput").ap(),
        "adj": nc.dram_tensor("adj", [N, N], I32, kind="ExternalInput").ap(),
        "node_type": nc.dram_tensor("node_type", [N, N], FP32,
                                    kind="ExternalInput").ap(),
        "level": nc.dram_tensor("level", [N], FP32, kind="ExternalInput").ap(),
        "W": nc.dram_tensor("W", [F_in, F_out], FP32, kind="ExternalInput").ap(),
        "a": nc.dram_tensor("a", [2 * F_out, 1], FP32, kind="ExternalInput").ap(),
    }
    out_ap = nc.dram_tensor("out", [N, F_out], FP32, kind="ExternalOutput").ap()
    with tile.TileContext(nc) as tc:
        attn_kernel(tc, out_ap, in_aps, cfg)
    nc.compile()
    return nc


_NC_CACHE = {}


def _get_nc(cfg: Cfg):
    key = (cfg.N, cfg.F_in, cfg.F_out)
    if key not in _NC_CACHE:
        _NC_CACHE[key] = build(cfg)
    return _NC_CACHE[key]


def run_on_cores(inputs: dict, cfg: Cfg, trace: bool = False):
    """Shard batch across cores, run, gather. Returns (out[B,N,F_out], bkr)."""
    from concourse.bass_utils import run_bass_kernel_spmd

    B = inputs["h"].shape[0]
    nc = _get_nc(cfg)
    in_maps = []
    for b in range(B):
        in_maps.append({
            "h": np.ascontiguousarray(inputs["h"][b], dtype=np.float32),
            "adj": np.ascontiguousarray(inputs["adj"][b], dtype=np.int32),
            "node_type": np.ascontiguousarray(inputs["node_type"][b],
                                              dtype=np.float32),
            "level": np.ascontiguousarray(inputs["level"][b], dtype=np.float32),
            "W": np.ascontiguousarray(inputs["W"], dtype=np.float32),
            "a": np.ascontiguousarray(inputs["a"], dtype=np.float32),
        })
    bkr = run_bass_kernel_spmd(nc, in_maps, list(range(B)), trace=trace)
    out = np.stack([bkr.results[b]["out"] for b in range(B)], axis=0)
    return out, bkr


def kernel(**inputs) -> np.ndarray:
    cfg = Cfg(N=2048, F_in=256, F_out=64)
    out, _ = run_on_cores(inputs, cfg, trace=False)
    return out.astype(np.float32)


if __name__ == "__main__":
    cfg = Cfg()
    nc = build(cfg)
    print("built ok")
